# revision 142
# baseline (speedup 1.0000x reference)
import math
from contextlib import ExitStack

import numpy as np
import ml_dtypes

import concourse.bass as bass
import concourse.tile as tile
from concourse import bacc, mybir
from concourse.bass_utils import run_bass_kernel_spmd

bf16 = ml_dtypes.bfloat16
F32 = mybir.dt.float32
BF = mybir.dt.bfloat16

B, S, Z, DOWN, UP, H, RHD, VHD = 2, 2048, 1024, 512, 1024, 16, 64, 64
HPC = 4
NCORES = 8
SCALE = 1.0 / (math.sqrt(64) + math.sqrt(64))

_cache = {}


def _rope_tables():
    theta = 1.0 / (10000.0 ** (np.arange(0, RHD, 2, dtype=np.float32) / RHD))
    pe = np.arange(S, dtype=np.float32)[:, None] * theta[None, :]
    cos_pos = np.repeat(np.sin(pe), 2, axis=-1).T.astype(np.float32)
    sin_pos = np.repeat(np.cos(pe), 2, axis=-1).T.astype(np.float32)
    return cos_pos, sin_pos


def _partner_cols(w):
    wp = np.empty_like(w)
    wp[..., 0::2] = -w[..., 1::2]
    wp[..., 1::2] = w[..., 0::2]
    return wp


def build_nc(with_bias):
    nc = bacc.Bacc("TRN2", target_bir_lowering=False, debug=False,
                   num_devices=NCORES)

    def din(name, shape, dt=BF):
        return nc.dram_tensor(name, shape, dt, kind="ExternalInput").ap()

    qT = din("qT", [Z, S])
    kT = din("kT", [Z, S])
    wdq = din("wdq", [Z, DOWN])
    wdkv = din("wdkv", [Z, DOWN])
    w1 = din("w1", [DOWN, 512])
    w2 = din("w2", [DOWN, 256])
    wk = din("wk", [DOWN, 256])
    wv = din("wv", [DOWN, 256])
    wkr2 = din("wkr2", [Z, 128])
    ct1 = din("ct1", [128, S])
    st1 = din("st1", [128, S])
    wfc = din("wfc", [256, Z])
    if with_bias:
        bdq = din("bdq", [128, 4], F32)
        bdkv = din("bdkv", [128, 4], F32)
        biasq = din("biasq", [512, S])
        biask = din("biask", [512, S])
    outT = nc.dram_tensor("outT", [Z, S], BF, kind="ExternalOutput").ap()

    with tile.TileContext(nc) as tc, ExitStack() as ctx:
        sp = ctx.enter_context(tc.tile_pool(name="static", bufs=1))

        def stile(shape, dt, name):
            return sp.tile(shape, dt, name=name, tag=name)

        wdq_sb = stile([128, 8, DOWN], BF, "wdq_sb")
        wdkv_sb = stile([128, 8, DOWN], BF, "wdkv_sb")
        w1_sb = stile([128, 4, 512], BF, "w1_sb")
        w2_sb = stile([128, 4, 256], BF, "w2_sb")
        wk_sb = stile([128, 4, 256], BF, "wk_sb")
        wv_sb = stile([128, 4, 256], BF, "wv_sb")
        wkr2_sb = stile([128, 8, 128], BF, "wkr2_sb")
        wfc_sb = stile([128, 2, 8, 128], BF, "wfc_sb")
        ct_sb = stile([128, S], BF, "ct_sb")
        st_sb = stile([128, S], BF, "st_sb")
        nc.sync.dma_start(wdq_sb[:], wdq.rearrange("(c p) m -> p c m", p=128))
        nc.sync.dma_start(wdkv_sb[:], wdkv.rearrange("(c p) m -> p c m", p=128))
        nc.sync.dma_start(wkr2_sb[:], wkr2.rearrange("(c p) m -> p c m", p=128))
        if with_bias:
            bdq_sb = stile([128, 4], F32, "bdq_sb")
            bdkv_sb = stile([128, 4], F32, "bdkv_sb")
            nc.sync.dma_start(bdq_sb[:], bdq[:])
            nc.sync.dma_start(bdkv_sb[:], bdkv[:])

        cq_sb = stile([128, 4, S], BF, "cq_sb")
        ckv_sb = stile([128, 4, S], BF, "ckv_sb")
        qcat_sb = stile([128, 4, S], BF, "qcat_sb")
        kcat_sb = stile([128, 4, S], BF, "kcat_sb")
        va_sb = stile([128, 16, HPC * 65], BF, "va_sb")
        af_sb = stile([128, 2, S], BF, "af_sb")
        tmpa_sb = stile([128, S], BF, "tmpa_sb")
        tmpb_sb = stile([128, S], BF, "tmpb_sb")

        pps = ctx.enter_context(tc.tile_pool(name="pps", bufs=2, space="PSUM"))

        def psA():
            return pps.tile([128, 1024], F32, name="psA", tag="psA")

        def psB():
            return pps.tile([128, 1024], F32, name="psB", tag="psB")

        with tc.tile_pool(name="qk_stream", bufs=1) as qkp:
            qT_sb = qkp.tile([128, 8, S], BF, name="qT_sb", tag="qT_sb")
            kT_sb = qkp.tile([128, 8, S], BF, name="kT_sb", tag="kT_sb")
            qT_r = qT.rearrange("(c p) s -> p c s", p=128)
            kT_r = kT.rearrange("(c p) s -> p c s", p=128)
            for sh in range(2):
                ssl = slice(1024 * sh, 1024 * (sh + 1))
                for zc in range(8):
                    nc.sync.dma_start(qT_sb[:, zc, ssl], qT_r[:, zc, ssl])
                for zc in range(8):
                    nc.sync.dma_start(kT_sb[:, zc, ssl], kT_r[:, zc, ssl])

            nc.sync.dma_start(w1_sb[:], w1.rearrange("(c p) m -> p c m", p=128))
            nc.sync.dma_start(w2_sb[:], w2.rearrange("(c p) m -> p c m", p=128))
            nc.sync.dma_start(wk_sb[:], wk.rearrange("(c p) m -> p c m", p=128))
            nc.sync.dma_start(wv_sb[:], wv.rearrange("(c p) m -> p c m", p=128))
            nc.sync.dma_start(ct_sb[:], ct1[:])
            nc.sync.dma_start(st_sb[:], st1[:])
            nc.sync.dma_start(wfc_sb[:],
                              wfc.rearrange("(c p) (z m) -> p c z m",
                                            p=128, m=128))

            for (src, wsb, dst, bcol) in (
                (qT_sb, wdq_sb, cq_sb, "q"),
                (kT_sb, wdkv_sb, ckv_sb, "kv"),
            ):
                for sf in range(4):
                    for m in range(4):
                        ps = (psA if (m + sf) % 2 == 0 else psB)()[:, 0:512]
                        for zc in range(8):
                            nc.tensor.matmul(
                                ps[:], wsb[:, zc, 128 * m:128 * (m + 1)],
                                src[:, zc, 512 * sf:512 * (sf + 1)],
                                start=(zc == 0), stop=(zc == 7))
                        dd = dst[:, m, 512 * sf:512 * (sf + 1)]
                        if not with_bias:
                            nc.scalar.copy(dd, ps[:])
                        elif with_bias:
                            bs = bdq_sb if bcol == "q" else bdkv_sb
                            nc.vector.tensor_tensor(
                                dd, ps[:], bs[:, m:m + 1].to_broadcast([128, 512]),
                                mybir.AluOpType.add)

            if True:
                for sf in range(4):
                    ps = psB()[:, 0:512]
                    for zc in range(8):
                        nc.tensor.matmul(
                            ps[:], wkr2_sb[:, zc, :],
                            kT_sb[:, zc, 512 * sf:512 * (sf + 1)],
                            start=(zc == 0), stop=(zc == 7))
                    nc.scalar.copy(tmpa_sb[:, 512 * sf:512 * (sf + 1)], ps[:])
            nc.sync.dma_start(tmpb_sb[64:128, :], tmpa_sb[0:64, :])

        if with_bias:
            bias_pool = ctx.enter_context(tc.tile_pool(name="bias_pool", bufs=1))
            biasq_sb = bias_pool.tile([128, 4, S], BF, name="biasq_sb", tag="biasq_sb")
            biask_sb = bias_pool.tile([128, 4, S], BF, name="biask_sb", tag="biask_sb")
            nc.sync.dma_start(biasq_sb[:], biasq.rearrange("(c p) s -> p c s", p=128))
            nc.sync.dma_start(biask_sb[:], biask.rearrange("(c p) s -> p c s", p=128))

        wrk3 = ctx.enter_context(tc.tile_pool(name="wrk3", bufs=3))
        va_v = va_sb.rearrange("p sc (h e) -> p sc h e", e=65)

        def attention(h):
            for qh in range(2):
                pv = psB()
                for kc in range(16):
                    sc_ps = psA()
                    for half in range(2):
                        psl = slice(512 * half, 512 * (half + 1))
                        rsl = slice(1024 * qh + 512 * half,
                                    1024 * qh + 512 * (half + 1))
                        nc.tensor.matmul(
                            sc_ps[:, psl],
                            kcat_sb[:, h, 128 * kc:128 * (kc + 1)],
                            qcat_sb[:, h, rsl], start=True, stop=True)
                    pr = wrk3.tile([128, 1024], BF, name="pr", tag="pr",
                                   bufs=3 if with_bias else 8)
                    nc.scalar.activation(pr[:], sc_ps[:],
                                         mybir.ActivationFunctionType.Exp,
                                         scale=SCALE)
                    for half in range(2):
                        psl = slice(512 * half, 512 * (half + 1))
                        nc.tensor.matmul(
                            pv[0:65, psl], va_v[:, kc, h, :], pr[:, psl],
                            start=(kc == 0), stop=(kc == 15))
                qsl = slice(1024 * qh, 1024 * (qh + 1))
                srow = wrk3.tile([1, 1024], F32, name="srow", tag="srow", bufs=2 if with_bias else 3)
                nc.vector.tensor_copy(srow[:], pv[64:65, :])
                rec = wrk3.tile([1, 1024], F32, name="rec", tag="rec")
                nc.vector.reciprocal_approx_fast(rec[:], srow[:])
                bc = wrk3.tile([64, 1024], F32, name="bc", tag="bc", bufs=2 if with_bias else 3)
                nc.gpsimd.partition_broadcast(bc[:], rec[:])
                ro = slice(0, 64) if h % 2 == 0 else slice(64, 128)
                nc.vector.tensor_tensor(af_sb[ro, h // 2, qsl],
                                        pv[0:64, :], bc[:],
                                        mybir.AluOpType.mult)

        with tc.tile_pool(name="wrk2", bufs=3) as wrk2:
            k0 = kcat_sb[64:128, 0, :]
            tt2 = wrk2.tile([128, S], BF, name="tt2", tag="tt2", bufs=1 if with_bias else 3)
            nc.vector.tensor_tensor(k0, tmpa_sb[64:128, :], ct_sb[64:128, :],
                                    mybir.AluOpType.mult)
            nc.vector.tensor_tensor(tt2[64:128, :], tmpb_sb[64:128, :],
                                    st_sb[64:128, :], mybir.AluOpType.mult)
            nc.vector.tensor_tensor(k0, k0, tt2[64:128, :], mybir.AluOpType.add)
            if with_bias:
                nc.vector.tensor_tensor(k0, k0, biask_sb[64:128, 0, :],
                                        mybir.AluOpType.add)
            for h in range(1, HPC):
                kd = kcat_sb[64:128, h, :]
                nc.vector.tensor_copy(kd, k0)
                if with_bias:
                    pass

            for sf in range(2):
                ssl = slice(1024 * sf, 1024 * (sf + 1))
                for pair in range(0, 1):
                    pb = psB()
                    for half in range(2):
                        hsl = slice(1024 * sf + 512 * half,
                                    1024 * sf + 512 * (half + 1))
                        psl = slice(512 * half, 512 * (half + 1))
                        for dc in range(4):
                            nc.tensor.matmul(
                                pb[:, psl],
                                w2_sb[:, dc, 128 * pair:128 * (pair + 1)],
                                cq_sb[:, dc, hsl], start=(dc == 0), stop=(dc == 3))
                    for sub in range(2):
                        h = 2 * pair + sub
                        rsl = slice(64 * sub, 64 * (sub + 1))
                        pa = psA()
                        for half in range(2):
                            hsl = slice(1024 * sf + 512 * half,
                                        1024 * sf + 512 * (half + 1))
                            psl = slice(512 * half, 512 * (half + 1))
                            for dc in range(4):
                                nc.tensor.matmul(
                                    pa[:, psl],
                                    w1_sb[:, dc, 128 * h:128 * (h + 1)],
                                    cq_sb[:, dc, hsl],
                                    start=(dc == 0), stop=(dc == 3))
                        qd = qcat_sb[:, h, ssl]
                        tt = wrk2.tile([128, 1024], BF, name="tt", tag="tt", bufs=2 if with_bias else 3)
                        nc.vector.tensor_tensor(qd, pa[:], ct_sb[:, ssl],
                                                mybir.AluOpType.mult)
                        nc.vector.tensor_tensor(
                            tt[64:128, :], pb[rsl, :], st_sb[rsl, ssl],
                            mybir.AluOpType.mult)
                        nc.vector.tensor_tensor(qd[64:128, :], qd[64:128, :],
                                                tt[64:128, :],
                                                mybir.AluOpType.add)
                        if with_bias:
                            nc.vector.tensor_tensor(qd, qd, biasq_sb[:, h, ssl],
                                                    mybir.AluOpType.add)

            for pair in range(0, 1):
                for sf in range(2):
                    ssl = slice(1024 * sf, 1024 * (sf + 1))
                    pk = psA()
                    for half in range(2):
                        hsl = slice(1024 * sf + 512 * half,
                                    1024 * sf + 512 * (half + 1))
                        psl = slice(512 * half, 512 * (half + 1))
                        for dc in range(4):
                            nc.tensor.matmul(
                                pk[:, psl],
                                wk_sb[:, dc, 128 * pair:128 * (pair + 1)],
                                ckv_sb[:, dc, hsl], start=(dc == 0), stop=(dc == 3))
                    for sub in range(2):
                        h = 2 * pair + sub
                        kd = kcat_sb[0:64, h, ssl]
                        psrc = pk[64 * sub:64 * (sub + 1), :]
                        if with_bias:
                            nc.vector.tensor_copy(kd, psrc)
                        else:
                            nc.scalar.copy(kd, psrc)
                        if with_bias:
                            nc.vector.tensor_tensor(kd, kd,
                                                    biask_sb[0:64, h, ssl],
                                                    mybir.AluOpType.add)

            nc.any.memset(va_sb[:, :, 64::65], 1.0)
            for sc in range(16):
                pv_ = psB()[:, 0:256]
                for dc in range(4):
                    nc.tensor.matmul(
                        pv_[:], ckv_sb[:, dc, 128 * sc:128 * (sc + 1)],
                        wv_sb[:, dc, :], start=(dc == 0), stop=(dc == 3))
                dst = va_sb[:, sc, :].rearrange("p (h e) -> p h e", e=65)[:, :, 0:64]
                nc.scalar.copy(dst, pv_[:].rearrange("p (h e) -> p h e", e=64))

            for sf in range(2):
                ssl = slice(1024 * sf, 1024 * (sf + 1))
                for pair in range(1, 2):
                    pb = psB()
                    for half in range(2):
                        hsl = slice(1024 * sf + 512 * half,
                                    1024 * sf + 512 * (half + 1))
                        psl = slice(512 * half, 512 * (half + 1))
                        for dc in range(4):
                            nc.tensor.matmul(
                                pb[:, psl],
                                w2_sb[:, dc, 128 * pair:128 * (pair + 1)],
                                cq_sb[:, dc, hsl], start=(dc == 0), stop=(dc == 3))
                    for sub in range(2):
                        h = 2 * pair + sub
                        rsl = slice(64 * sub, 64 * (sub + 1))
                        pa = psA()
                        for half in range(2):
                            hsl = slice(1024 * sf + 512 * half,
                                        1024 * sf + 512 * (half + 1))
                            psl = slice(512 * half, 512 * (half + 1))
                            for dc in range(4):
                                nc.tensor.matmul(
                                    pa[:, psl],
                                    w1_sb[:, dc, 128 * h:128 * (h + 1)],
                                    cq_sb[:, dc, hsl],
                                    start=(dc == 0), stop=(dc == 3))
                        qd = qcat_sb[:, h, ssl]
                        tt = wrk2.tile([128, 1024], BF, name="tt", tag="tt", bufs=2 if with_bias else 3)
                        nc.vector.tensor_tensor(qd, pa[:], ct_sb[:, ssl],
                                                mybir.AluOpType.mult)
                        nc.vector.tensor_tensor(
                            tt[64:128, :], pb[rsl, :], st_sb[rsl, ssl],
                            mybir.AluOpType.mult)
                        nc.vector.tensor_tensor(qd[64:128, :], qd[64:128, :],
                                                tt[64:128, :],
                                                mybir.AluOpType.add)
                        if with_bias:
                            nc.vector.tensor_tensor(qd, qd, biasq_sb[:, h, ssl],
                                                    mybir.AluOpType.add)

            for pair in range(1, 2):
                for sf in range(2):
                    ssl = slice(1024 * sf, 1024 * (sf + 1))
                    pk = psA()
                    for half in range(2):
                        hsl = slice(1024 * sf + 512 * half,
                                    1024 * sf + 512 * (half + 1))
                        psl = slice(512 * half, 512 * (half + 1))
                        for dc in range(4):
                            nc.tensor.matmul(
                                pk[:, psl],
                                wk_sb[:, dc, 128 * pair:128 * (pair + 1)],
                                ckv_sb[:, dc, hsl], start=(dc == 0), stop=(dc == 3))
                    for sub in range(2):
                        h = 2 * pair + sub
                        kd = kcat_sb[0:64, h, ssl]
                        psrc = pk[64 * sub:64 * (sub + 1), :]
                        if with_bias:
                            nc.vector.tensor_copy(kd, psrc)
                        else:
                            nc.scalar.copy(kd, psrc)
                        if with_bias:
                            nc.vector.tensor_tensor(kd, kd,
                                                    biask_sb[0:64, h, ssl],
                                                    mybir.AluOpType.add)

            attention(0)
            attention(1)
            attention(2)
            attention(3)

        with tc.tile_pool(name="wrk4", bufs=4) as wrk4:
            for qf in range(4):
                for zc in range(8):
                    qsl = slice(512 * qf, 512 * (qf + 1))
                    fp = (psA if zc % 2 == 0 else psB)()[:, 0:512]
                    for c in range(2):
                        nc.tensor.matmul(fp[:], wfc_sb[:, c, zc, :],
                                         af_sb[:, c, qsl],
                                         start=(c == 0), stop=(c == 1))
                    ob = wrk4.tile([128, 512], F32, name="ob", tag="ob", bufs=3 if with_bias else 4)
                    if zc % 2 == 0:
                        nc.vector.tensor_copy(ob[:], fp[:])
                    else:
                        nc.scalar.copy(ob[:], fp[:])
                    nc.sync.dma_start(outT[128 * zc:128 * (zc + 1), qsl], ob[:])

    nc.compile()
    return nc


def _prep_in_maps(inputs):
    f32 = np.float32
    q = np.asarray(inputs["query"], f32)
    k = np.asarray(inputs["key"], f32)
    w_dq = np.asarray(inputs["w_dq"], f32)
    w_dkv = np.asarray(inputs["w_dkv"], f32)
    w_uq = np.asarray(inputs["w_uq"], f32)
    w_uk = np.asarray(inputs["w_uk"], f32)
    w_uv = np.asarray(inputs["w_uv"], f32)
    w_qr = np.asarray(inputs["w_qr"], f32)
    w_kr = np.asarray(inputs["w_kr"], f32)
    w_fc = np.asarray(inputs["w_fc"], f32)
    b_dq = np.asarray(inputs["b_dq"], f32)
    b_dkv = np.asarray(inputs["b_dkv"], f32)
    b_uq = np.asarray(inputs["b_uq"], f32)
    b_uk = np.asarray(inputs["b_uk"], f32)
    b_qr = np.asarray(inputs["b_qr"], f32)
    b_kr = np.asarray(inputs["b_kr"], f32)

    CT, ST = _rope_tables()
    ct1 = np.concatenate([np.ones((64, S), f32), CT], axis=0)
    st1 = np.concatenate([ST, ST], axis=0)

    with_bias = any(np.any(np.asarray(inputs[n])) for n in
                    ("b_dq", "b_dkv", "b_uq", "b_uk", "b_qr", "b_kr"))

    qTb = [q[b_].T.astype(bf16) for b_ in range(B)]
    kTb = [k[b_].T.astype(bf16) for b_ in range(B)]

    in_maps = []
    for core in range(NCORES):
        b_idx, grp = core // HPC, core % HPC
        h0 = HPC * grp
        hsl = slice(64 * h0, 64 * (h0 + HPC))
        W1 = np.zeros((DOWN, 512), f32)
        W2 = np.zeros((DOWN, 256), f32)
        Wk = np.zeros((DOWN, 256), f32)
        for i in range(HPC):
            hh = h0 + i
            W1[:, 128 * i:128 * i + 64] = w_uq[:, 64 * hh:64 * hh + 64]
            W1[:, 128 * i + 64:128 * (i + 1)] = w_qr[:, 64 * hh:64 * hh + 64]
            W2[:, 64 * i:64 * (i + 1)] = _partner_cols(
                w_qr[:, 64 * hh:64 * hh + 64])
            Wk[:, 64 * i:64 * (i + 1)] = w_uk[:, 64 * hh:64 * hh + 64]
        m = {
            "qT": qTb[b_idx], "kT": kTb[b_idx],
            "wdq": w_dq.astype(bf16), "wdkv": w_dkv.astype(bf16),
            "w1": W1.astype(bf16), "w2": W2.astype(bf16),
            "wk": Wk.astype(bf16), "wv": w_uv[:, hsl].astype(bf16),
            "wkr2": np.concatenate([_partner_cols(w_kr), w_kr],
                                   axis=1).astype(bf16),
            "ct1": ct1.astype(bf16), "st1": st1.astype(bf16),
            "wfc": w_fc[hsl, :].astype(bf16),
        }
        if with_bias:
            bq = np.zeros((512, S), f32)
            bk = np.zeros((512, S), f32)
            for i in range(HPC):
                hh = h0 + i
                bq[128 * i:128 * i + 64] = b_uq[64 * hh:64 * hh + 64, None]
                bq[128 * i + 64:128 * (i + 1)] = (
                    b_qr[64 * hh:64 * hh + 64, None] * CT
                    + _partner_cols(b_qr[None, 64 * hh:64 * hh + 64])[0][:, None] * ST)
                bk[128 * i:128 * i + 64] = b_uk[64 * hh:64 * hh + 64, None]
                bk[128 * i + 64:128 * (i + 1)] = (
                    b_kr[:, None] * CT
                    + _partner_cols(b_kr[None, :])[0][:, None] * ST)
            m["bdq"] = b_dq.reshape(4, 128).T.copy()
            m["bdkv"] = b_dkv.reshape(4, 128).T.copy()
            m["biasq"] = bq.astype(bf16)
            m["biask"] = bk.astype(bf16)
        in_maps.append(m)
    return in_maps, with_bias


e4m3 = ml_dtypes.float8_e4m3
FP8 = mybir.dt.float8e4
I32 = mybir.dt.int32
DR = mybir.MatmulPerfMode.DoubleRow
FSCALE = 1.0 / 16.0
SCH_A = float((1 << 23) / math.log(2.0) / 16.0)
SCH_B = float(127.0 * (1 << 23) - 486408.0)
SCH_KC = (9, 13)
SCH_KC_LAST = (5, 7, 9, 11, 13, 15)


def build_fast():
    nc = bacc.Bacc("TRN2", target_bir_lowering=False, debug=False,
                   num_devices=NCORES)

    def din(name, shape, dt):
        return nc.dram_tensor(name, shape, dt, kind="ExternalInput").ap()

    q8d = din("q8", [Z, S], FP8)
    kTd = din("kT", [Z, S], BF)
    k8d = din("k8", [Z, S], FP8)
    wdq8 = din("wdq8", [Z, DOWN], FP8)
    wdkv = din("wdkv", [Z, DOWN], BF)
    wkr48 = din("wkr48", [Z, 256], FP8)
    wq8 = din("wq8", [DOWN, 768], FP8)
    wk = din("wk", [DOWN, 256], BF)
    wv = din("wv", [DOWN, 256], BF)
    ct2 = din("ct2", [128, S], BF)
    st2 = din("st2", [128, S], BF)
    wfc = din("wfc", [256, Z], BF)
    outT = nc.dram_tensor("outT", [Z, S], BF, kind="ExternalOutput").ap()

    with tile.TileContext(nc) as tc, ExitStack() as ctx:
        wt = ctx.enter_context(tc.tile_pool(name="wt", bufs=1))

        def wtile(shape, dt, name):
            return wt.tile(shape, dt, name=name, tag=name)

        wdq8_sb = wtile([128, 4, 2, DOWN], FP8, "wdq8_sb")
        wdkv_sb = wtile([128, 8, DOWN], BF, "wdkv_sb")
        wkr48_sb = wtile([128, 4, 2, 256], FP8, "wkr48_sb")
        wq8_sb = wtile([128, 2, 2, 768], FP8, "wq8_sb")
        wk_sb = wtile([128, 4, 256], BF, "wk_sb")
        wv_sb = wtile([128, 4, 256], BF, "wv_sb")
        ct_sb = wtile([128, S], BF, "ct_sb")
        st_sb = wtile([128, S], BF, "st_sb")
        wfc_sb = wtile([128, 2, 8, 128], BF, "wfc_sb")



        act1 = ctx.enter_context(tc.tile_pool(name="act1", bufs=1))

        def atile(shape, dt, name):
            return act1.tile(shape, dt, name=name, tag=name)

        cq8_sb = atile([128, 2, 2, S], FP8, "cq8_sb")
        ckv_sb = atile([128, 4, S], BF, "ckv_sb")
        qcat8 = [atile([128, 2, S], FP8, f"qcat8_{p}") for p in range(2)]
        kcat8 = [atile([128, 2, S], FP8, f"kcat8_{p}") for p in range(2)]
        va_sb = atile([128, 16, 4 * 65], BF, "va_sb")
        af_sb = atile([128, 2, 16, 128], BF, "af_sb")
        afT_sb = atile([128, 2, S], BF, "afT_sb")

        wrk = ctx.enter_context(tc.tile_pool(name="wrk", bufs=2))
        stp = ctx.enter_context(tc.tile_pool(name="stream", bufs=2))
        prp = ctx.enter_context(tc.tile_pool(name="prp", bufs=2))

        pp = ctx.enter_context(tc.tile_pool(name="pp", bufs=2, space="PSUM"))

        def ps_big():
            return pp.tile([128, 1024], F32, name="ps_big", tag="ps_big",
                           bufs=3)

        def ps_small():
            return pp.tile([128, 512], F32, name="ps_small", tag="ps_small")

        def ps_pv():
            return pp.tile([128, 260], F32, name="ps_pv", tag="ps_pv")

        nc.any.memset(va_sb[:, :, 64::65], 1.0)

        q8_r = q8d.rearrange("(zp two p) s -> p zp two s", p=128, two=2)
        kT_r = kTd.rearrange("(zc p) s -> p zc s", p=128)
        k8_r = k8d.rearrange("(zp two p) s -> p zp two s", p=128, two=2)
        va_v = va_sb.rearrange("k kc (h e) -> k kc h e", e=65)

        def emit_kside(sf):
            ssl = slice(512 * sf, 512 * (sf + 1))
            kt = stp.tile([128, 8, 512], BF, name="kt", tag="kt")
            for zc in range(8):
                nc.sync.dma_start(kt[:, zc, :], kT_r[:, zc, ssl])
            k8t = stp.tile([128, 4, 2, 512], FP8, name="k8t", tag="k8t")
            for zp in range(4):
                nc.sync.dma_start(k8t[:, zp, :, :], k8_r[:, zp, :, ssl])
            for m in range(4):
                ps = ps_small()
                for zc in range(8):
                    nc.tensor.matmul(
                        ps[:], wdkv_sb[:, zc, 128 * m:128 * (m + 1)],
                        kt[:, zc, :], start=(zc == 0), stop=(zc == 7))
                (nc.scalar.copy if sf < 2 else nc.vector.tensor_copy)(
                    ckv_sb[:, m, ssl], ps[:])
            pb = ps_big()
            for half in range(2):
                for zp in range(4):
                    nc.tensor.matmul(
                        pb[:, 512 * half:512 * (half + 1)],
                        wkr48_sb[:, zp, :, 128 * half:128 * (half + 1)],
                        k8t[:, zp, :, :],
                        start=(zp == 0), stop=(zp == 3), perf_mode=DR)
            kr1 = wrk.tile([128, 512], BF, name="kr1", tag="kr1", bufs=3)
            kr2 = wrk.tile([128, 512], BF, name="kr2", tag="kr2", bufs=3)
            with tc.high_priority():
                nc.vector.tensor_tensor(kr1[64:128, :], pb[64:128, 0:512],
                                        ct_sb[64:128, ssl], mybir.AluOpType.mult)
                nc.vector.tensor_tensor(kr1[0:64, :], pb[0:64, 512:1024],
                                        ct_sb[0:64, ssl], mybir.AluOpType.mult)
                nc.vector.tensor_tensor(kr2[64:128, :], pb[64:128, 512:1024],
                                        st_sb[64:128, ssl], mybir.AluOpType.mult)
                nc.vector.tensor_tensor(kr2[0:64, :], pb[0:64, 0:512],
                                        st_sb[0:64, ssl], mybir.AluOpType.mult)
                nc.gpsimd.tensor_tensor(kcat8[0][:, 1, ssl], kr1[:], kr2[:],
                                        mybir.AluOpType.add)
                nc.scalar.dma_start(kcat8[1][:, 1, ssl], kcat8[0][:, 1, ssl])
            for p in range(2):
                ps = ps_small()
                for dc in range(4):
                    nc.tensor.matmul(
                        ps[:], wk_sb[:, dc, 128 * p:128 * (p + 1)],
                        ckv_sb[:, dc, ssl], start=(dc == 0), stop=(dc == 3))
                with tc.high_priority():
                    (nc.scalar.copy if sf < 2 else nc.vector.tensor_copy)(
                        kcat8[p][:, 0, ssl], ps[:])

        def emit_qside(sf):
            ssl = slice(512 * sf, 512 * (sf + 1))
            qt = stp.tile([128, 4, 2, 512], FP8, name="qt", tag="qt")
            for zp in range(4):
                nc.sync.dma_start(qt[:, zp, :, :], q8_r[:, zp, :, ssl])
            for m in range(4):
                ps = ps_small()
                for zp in range(4):
                    nc.tensor.matmul(
                        ps[:], wdq8_sb[:, zp, :, 128 * m:128 * (m + 1)],
                        qt[:, zp, :, :],
                        start=(zp == 0), stop=(zp == 3), perf_mode=DR)
                (nc.scalar.copy if sf < 2 else nc.vector.tensor_copy)(
                    cq8_sb[:, m // 2, m % 2, ssl], ps[:])
            for p in range(2):
                ps = ps_small()
                for dp in range(2):
                    nc.tensor.matmul(
                        ps[:], wq8_sb[:, dp, :, 384 * p:384 * p + 128],
                        cq8_sb[:, dp, :, ssl],
                        start=(dp == 0), stop=(dp == 1), perf_mode=DR)
                (nc.scalar.copy if sf < 2 else nc.vector.tensor_copy)(
                    qcat8[p][:, 0, ssl], ps[:])
                pb = ps_big()
                for dp in range(2):
                    nc.tensor.matmul(
                        pb[:, 0:512], wq8_sb[:, dp, :, 384 * p + 128:384 * p + 256],
                        cq8_sb[:, dp, :, ssl],
                        start=(dp == 0), stop=(dp == 1), perf_mode=DR)
                for dp in range(2):
                    nc.tensor.matmul(
                        pb[:, 512:1024], wq8_sb[:, dp, :, 384 * p + 256:384 * p + 384],
                        cq8_sb[:, dp, :, ssl],
                        start=(dp == 0), stop=(dp == 1), perf_mode=DR)
                t1 = wrk.tile([128, 512], BF, name="t1", tag="t1", bufs=1)
                t2 = wrk.tile([128, 512], BF, name="t2", tag="t2", bufs=1)
                nc.vector.tensor_tensor(t1[:], pb[:, 0:512], ct_sb[:, ssl],
                                        mybir.AluOpType.mult)
                nc.vector.tensor_tensor(t2[:], pb[:, 512:1024], st_sb[:, ssl],
                                        mybir.AluOpType.mult)
                nc.gpsimd.tensor_tensor(qcat8[p][:, 1, ssl], t1[:], t2[:],
                                        mybir.AluOpType.add)

        def emit_v(sf):
            for kc in range(4 * sf, 4 * sf + 4):
                ps = ps_small()
                for dc in range(4):
                    nc.tensor.matmul(
                        ps[:, 0:256], ckv_sb[:, dc, 128 * kc:128 * (kc + 1)],
                        wv_sb[:, dc, :], start=(dc == 0), stop=(dc == 3))
                nc.any.tensor_copy(
                    va_v[:, kc, :, 0:64],
                    ps[:, 0:256].rearrange("k (h e) -> k h e", e=64))

        pr_tiles = {}

        def emit_scores(h, qb, kcs):
            pair, sub = h // 2, h % 2
            rows = slice(64 * sub, 64 * (sub + 1))
            if (h, qb) not in pr_tiles:
                pr_tiles[(h, qb)] = prp.tile([128, 16, 1024], BF, name="pr",
                                             tag="pr")
            pr = pr_tiles[(h, qb)]
            sch = SCH_KC_LAST if (h, qb) == (3, 1) else SCH_KC
            for kc in kcs:
                if kc in sch:
                    for half in range(2):
                        sp_ = ps_small()
                        nc.tensor.matmul(
                            sp_[:],
                            kcat8[pair][rows, :, 128 * kc:128 * (kc + 1)],
                            qcat8[pair][rows, :, slice(1024 * qb + 512 * half,
                                                       1024 * qb + 512 * (half + 1))],
                            start=True, stop=True, perf_mode=DR)
                        nc.vector.tensor_scalar(sp_[:].bitcast(I32), sp_[:],
                                                SCH_A, SCH_B,
                                                mybir.AluOpType.mult,
                                                mybir.AluOpType.add)
                        nc.vector.tensor_copy(
                            pr[:, kc, 512 * half:512 * (half + 1)], sp_[:])
                    continue
                sc = ps_big()
                for half in range(2):
                    nc.tensor.matmul(
                        sc[:, 512 * half:512 * (half + 1)],
                        kcat8[pair][rows, :, 128 * kc:128 * (kc + 1)],
                        qcat8[pair][rows, :, slice(1024 * qb + 512 * half,
                                                   1024 * qb + 512 * (half + 1))],
                        start=True, stop=True, perf_mode=DR)
                nc.scalar.activation(pr[:, kc, :], sc[:],
                                     mybir.ActivationFunctionType.Exp,
                                     scale=FSCALE)

        def emit_pv(h, qb):
            pair, sub = h // 2, h % 2
            pr = pr_tiles.pop((h, qb))
            for qc in range(8):
                pv = ps_pv()[:, 0:65]
                for kc in range(16):
                    nc.tensor.matmul(pv[:], pr[:, kc, 128 * qc:128 * (qc + 1)],
                                     va_v[:, kc, h, :],
                                     start=(kc == 0), stop=(kc == 15))
                rec = wrk.tile([128, 1], F32, name="rec", tag="rec", bufs=8)
                nc.vector.reciprocal_approx_fast(rec[:], pv[:, 64:65])
                qcg = 8 * qb + qc
                nc.any.tensor_scalar(
                    af_sb[:, pair, qcg, 64 * sub:64 * (sub + 1)], pv[:, 0:64],
                    rec[:], None, mybir.AluOpType.mult)
                if sub == 1:
                    nc.scalar.dma_start_transpose(
                        afT_sb[:, pair, 128 * qcg:128 * (qcg + 1)],
                        af_sb[:, pair, qcg, :])

        def emit_tail(qb):
            for zc in range(8):
                ob = wrk.tile([128, 2, 512], BF, name="ob", tag="ob", bufs=6)
                for qf in range(2):
                    qsl = slice(1024 * qb + 512 * qf, 1024 * qb + 512 * (qf + 1))
                    if qb == 1 and (zc + qf) % 2 == 1:
                        fp = ps_big()[:, 0:512]
                    else:
                        fp = ps_small()
                    for c in range(2):
                        nc.tensor.matmul(fp[:], wfc_sb[:, c, zc, :],
                                         afT_sb[:, c, qsl],
                                         start=(c == 0), stop=(c == 1))
                    nc.any.tensor_copy(ob[:, qf, :], fp[:])
                nc.sync.dma_start(
                    outT[128 * zc:128 * (zc + 1), 1024 * qb:1024 * (qb + 1)],
                    ob[:])

        wm = wt.tile([128, 512], BF, name="wm", tag="wm")
        nc.gpsimd.memset(wm[:], 1.0)
        wq8_r = wq8.rearrange("(zp two p) m -> p zp two m", p=128, two=2)
        load_wkv(slice(0, 128))
        load_k(0)
        load_q(0)
        load_q(1)
        nc.sync.dma_start(wkr8_sb[:], wkr8.rearrange("(zp two p) m -> p zp two m", p=128, two=2))
        load_ctst(0)
        load_wkv(slice(256, 512))
        nc.sync.dma_start(wq8_sb[:, :, :, 0:384], wq8_r[:, :, :, 0:384])
        for _w in range(8):
            wps = ps_big()
            nc.tensor.matmul(wps[:, 0:512], wm[:, 0:128], wm[:],
                             start=True, stop=True)
        emit_kcat(0, 0)
        emit_krope(0)
        emit_v(0, 0)
        emit_v(0, 1)
        emit_v(0, 2)
        emit_v(0, 3)
        emit_q(0, 0)
        emit_q(1, 0)
        load_k(1)
        load_ctst(1)

        def sc2(kc):
            emit_scores(0, 0, (kc,))
            emit_scores(1, 0, (kc,))

        sc2(0)
        sc2(1)
        load_k(2)
        load_ctst(2)
        sc2(2)
        sc2(3)
        emit_kcat(1, 0)
        sc2(4)
        emit_krope(1)
        load_k(3)
        load_ctst(3)
        sc2(5)
        emit_v(1, 0)
        sc2(6)
        emit_v(1, 1)
        nc.sync.dma_start(wq8_sb[:, :, :, 384:768], wq8_r[:, :, :, 384:768])
        load_wkv(slice(128, 256))
        sc2(7)
        emit_v(1, 2)
        emit_v(1, 3)
        emit_kcat(2, 0)
        sc2(8)
        emit_krope(2)
        load_q(2)
        load_q(3)
        sc2(9)
        emit_v(2, 0)
        sc2(10)
        emit_v(2, 1)
        nc.sync.dma_start(wfc_sb[:], wfc.rearrange("(c p) (zc m) -> p c zc m", p=128, m=128))
        sc2(11)
        emit_v(2, 2)
        emit_v(2, 3)
        emit_kcat(3, 0)
        sc2(12)
        emit_krope(3)
        sc2(13)
        emit_v(3, 0)
        emit_v(3, 1)
        sc2(14)
        emit_v(3, 2)
        emit_v(3, 3)
        emit_kcat(0, 1)
        sc2(15)
        emit_kcat(1, 1)
        emit_q(0, 1)
        emit_q(1, 1)
        emit_pv(0, 0, 0)
        emit_kcat(2, 1)
        emit_pv(0, 0, 1)
        emit_kcat(3, 1)
        emit_scores(2, 0, range(0, 4))
        emit_q(2, 0)
        emit_scores(2, 0, range(4, 8))
        emit_q(3, 0)
        emit_scores(2, 0, range(8, 12))
        emit_q(2, 1)
        emit_scores(2, 0, range(12, 16))
        emit_q(3, 1)
        emit_pv(1, 0, 0)
        emit_pv(1, 0, 1)
        emit_scores(3, 0, range(0, 8))
        emit_pv(2, 0, 0)
        emit_scores(3, 0, range(8, 16))
        emit_pv(2, 0, 1)
        emit_scores(0, 1, range(0, 8))
        emit_pv(3, 0, 0)
        emit_scores(0, 1, range(8, 16))
        emit_pv(3, 0, 1)
        emit_scores(1, 1, range(0, 8))
        emit_fc(0, 0, range(0, 4))
        emit_scores(1, 1, range(8, 16))
        emit_fc(0, 0, range(4, 8))
        emit_pv(0, 1, 0)
        emit_fc(0, 1, range(0, 4))
        emit_scores(2, 1, range(0, 8))
        emit_pv(0, 1, 1)
        emit_fc(0, 1, range(4, 8))
        emit_scores(2, 1, range(8, 16))
        emit_pv(1, 1, 0)
        emit_pv(1, 1, 1)
        emit_scores(3, 1, range(0, 8))
        emit_pv(2, 1, 0)
        emit_scores(3, 1, range(8, 16))
        emit_pv(3, 1, 0)
        emit_fc(1, 0, range(0, 8))
        emit_pv(2, 1, 1)
        emit_pv(3, 1, 1)
        emit_fc(1, 1, range(0, 8))

    nc.compile()
    return nc


I16 = mybir.dt.int16
SCH_A16 = SCH_A / (1 << 16)
SCH_B16 = SCH_B / (1 << 16) + 0.5


def build_fast2():
    nc = bacc.Bacc("TRN2", target_bir_lowering=False, debug=False,
                   num_devices=NCORES)

    def din(name, shape, dt):
        return nc.dram_tensor(name, shape, dt, kind="ExternalInput").ap()

    q8d = din("q8", [Z, S], FP8)
    kTd = din("kT", [Z, S], BF)
    k8d = din("k8", [Z, S], FP8)
    wkv = din("wkv", [Z, 512], BF)
    wq8 = din("wq8", [Z, 768], FP8)
    wkr8 = din("wkr8", [Z, 256], FP8)
    ct2 = din("ct2", [128, S], BF)
    st2 = din("st2", [128, S], BF)
    wfc = din("wfc", [256, Z], BF)
    outT = nc.dram_tensor("outT", [Z, S], BF, kind="ExternalOutput").ap()

    with tile.TileContext(nc) as tc, ExitStack() as ctx:
        wt = ctx.enter_context(tc.tile_pool(name="wt", bufs=1))

        def wtile(shape, dt, name):
            return wt.tile(shape, dt, name=name, tag=name)

        wkv_sb = wtile([128, 8, 512], BF, "wkv_sb")
        wq8_sb = wtile([128, 4, 2, 768], FP8, "wq8_sb")
        wkr8_sb = wtile([128, 4, 2, 256], FP8, "wkr8_sb")
        ct_sb = wtile([128, S], BF, "ct_sb")
        st_sb = wtile([128, S], BF, "st_sb")
        wfc_sb = wtile([128, 2, 8, 128], BF, "wfc_sb")

        act1 = ctx.enter_context(tc.tile_pool(name="act1", bufs=1))

        def atile(shape, dt, name):
            return act1.tile(shape, dt, name=name, tag=name)

        qcat8 = [atile([128, 2, S], FP8, f"qcat8_{p}") for p in range(2)]
        kcat8 = [atile([128, 2, S], FP8, f"kcat8_{p}") for p in range(2)]
        va_sb = atile([128, 16, 4 * 65], BF, "va_sb")
        af_sb = atile([128, 2, 16, 128], BF, "af_sb")
        afT_sb = atile([128, 2, S], BF, "afT_sb")

        wrk = ctx.enter_context(tc.tile_pool(name="wrk", bufs=2))
        stp = ctx.enter_context(tc.tile_pool(name="stream", bufs=2))
        prp = ctx.enter_context(tc.tile_pool(name="prp", bufs=2))

        pp = ctx.enter_context(tc.tile_pool(name="pp", bufs=2, space="PSUM"))

        def ps_big():
            return pp.tile([128, 1024], F32, name="ps_big", tag="ps_big",
                           bufs=3)

        def ps_small():
            return pp.tile([128, 512], F32, name="ps_small", tag="ps_small")

        def ps_pv():
            return ps_small()

        nc.any.memset(va_sb[:, :, 64::65], 1.0)

        q8_r = q8d.rearrange("(zp two p) s -> p zp two s", p=128, two=2)
        kT_r = kTd.rearrange("(zc p) s -> p zc s", p=128)
        k8_r = k8d.rearrange("(zp two p) s -> p zp two s", p=128, two=2)
        va_v = va_sb.rearrange("k kc (h e) -> k kc h e", e=65)

        def emit_kside(sf):
            ssl = slice(512 * sf, 512 * (sf + 1))
            kt = stp.tile([128, 8, 512], BF, name="kt", tag="kt")
            for zc in range(8):
                nc.sync.dma_start(kt[:, zc, :], kT_r[:, zc, ssl])
            k8t = stp.tile([128, 4, 2, 512], FP8, name="k8t", tag="k8t")
            for zp in range(4):
                nc.sync.dma_start(k8t[:, zp, :, :], k8_r[:, zp, :, ssl])
            for p in range(2):
                ps = ps_small()
                for zc in range(8):
                    nc.tensor.matmul(
                        ps[:], wkv_sb[:, zc, 128 * p:128 * (p + 1)],
                        kt[:, zc, :], start=(zc == 0), stop=(zc == 7))
                nc.any.tensor_copy(kcat8[p][:, 0, ssl], ps[:])
            pb = ps_big()
            for half in range(2):
                for zp in range(4):
                    nc.tensor.matmul(
                        pb[:, 512 * half:512 * (half + 1)],
                        wkr8_sb[:, zp, :, 128 * half:128 * (half + 1)],
                        k8t[:, zp, :, :],
                        start=(zp == 0), stop=(zp == 3), perf_mode=DR)
            kr1 = wrk.tile([128, 512], BF, name="kr1", tag="kr1", bufs=3)
            kr2 = wrk.tile([128, 512], BF, name="kr2", tag="kr2", bufs=3)
            nc.vector.tensor_tensor(kr1[:], pb[:, 0:512], ct_sb[:, ssl],
                                    mybir.AluOpType.mult)
            nc.vector.tensor_tensor(kr2[:], pb[:, 512:1024], st_sb[:, ssl],
                                    mybir.AluOpType.mult)
            nc.gpsimd.tensor_tensor(kcat8[0][:, 1, ssl], kr1[:], kr2[:],
                                    mybir.AluOpType.add)
            nc.sync.dma_start(kcat8[1][:, 1, ssl], kcat8[0][:, 1, ssl])
            for kc in range(4 * sf, 4 * sf + 4):
                ps = ps_small()
                for zc in range(8):
                    nc.tensor.matmul(
                        ps[:, 0:256], kt[:, zc, 128 * (kc % 4):128 * (kc % 4) + 128],
                        wkv_sb[:, zc, 256:512], start=(zc == 0), stop=(zc == 7))
                nc.any.tensor_copy(
                    va_v[:, kc, :, 0:64],
                    ps[:, 0:256].rearrange("k (h e) -> k h e", e=64))

        def emit_qside(sf):
            ssl = slice(512 * sf, 512 * (sf + 1))
            qt = stp.tile([128, 4, 2, 512], FP8, name="qt", tag="qt")
            for zp in range(4):
                nc.sync.dma_start(qt[:, zp, :, :], q8_r[:, zp, :, ssl])
            for p in range(2):
                ps = ps_small()
                for zp in range(4):
                    nc.tensor.matmul(
                        ps[:], wq8_sb[:, zp, :, 384 * p:384 * p + 128],
                        qt[:, zp, :, :],
                        start=(zp == 0), stop=(zp == 3), perf_mode=DR)
                nc.any.tensor_copy(qcat8[p][:, 0, ssl], ps[:])
                pb = ps_big()
                for zp in range(4):
                    nc.tensor.matmul(
                        pb[:, 0:512], wq8_sb[:, zp, :, 384 * p + 128:384 * p + 256],
                        qt[:, zp, :, :],
                        start=(zp == 0), stop=(zp == 3), perf_mode=DR)
                for zp in range(4):
                    nc.tensor.matmul(
                        pb[:, 512:1024], wq8_sb[:, zp, :, 384 * p + 256:384 * p + 384],
                        qt[:, zp, :, :],
                        start=(zp == 0), stop=(zp == 3), perf_mode=DR)
                t1 = wrk.tile([128, 512], BF, name="t1", tag="t1", bufs=3)
                t2 = wrk.tile([128, 512], BF, name="t2", tag="t2", bufs=3)
                nc.vector.tensor_tensor(t1[:], pb[:, 0:512], ct_sb[:, ssl],
                                        mybir.AluOpType.mult)
                nc.vector.tensor_tensor(t2[:], pb[:, 512:1024], st_sb[:, ssl],
                                        mybir.AluOpType.mult)
                nc.gpsimd.tensor_tensor(qcat8[p][:, 1, ssl], t1[:], t2[:],
                                        mybir.AluOpType.add)

        pr_tiles = {}

        def emit_scores(h, qb, kcs, dve_kcs):
            pair, sub = h // 2, h % 2
            rows = slice(64 * sub, 64 * (sub + 1))
            if (h, qb) not in pr_tiles:
                pr_tiles[(h, qb)] = prp.tile([128, 16, 1024], BF, name="pr",
                                             tag="pr")
            pr = pr_tiles[(h, qb)]
            for kc in kcs:
                sc = ps_big()
                for half in range(2):
                    nc.tensor.matmul(
                        sc[:, 512 * half:512 * (half + 1)],
                        kcat8[pair][rows, :, 128 * kc:128 * (kc + 1)],
                        qcat8[pair][rows, :, slice(1024 * qb + 512 * half,
                                                   1024 * qb + 512 * (half + 1))],
                        start=True, stop=True, perf_mode=DR)
                if kc in dve_kcs:
                    nc.any.tensor_scalar(pr[:, kc, :].bitcast(I16), sc[:],
                                         SCH_A16, SCH_B16,
                                         mybir.AluOpType.mult,
                                         mybir.AluOpType.add)
                else:
                    nc.scalar.activation(pr[:, kc, :], sc[:],
                                         mybir.ActivationFunctionType.Exp,
                                         scale=FSCALE)

        def emit_pv(h, qb):
            pair, sub = h // 2, h % 2
            pr = pr_tiles.pop((h, qb))
            for qc in range(8):
                pv = ps_pv()[:, 0:65]
                for kc in range(16):
                    nc.tensor.matmul(pv[:], pr[:, kc, 128 * qc:128 * (qc + 1)],
                                     va_v[:, kc, h, :],
                                     start=(kc == 0), stop=(kc == 15))
                rec = wrk.tile([128, 1], F32, name="rec", tag="rec", bufs=8)
                nc.vector.reciprocal_approx_fast(rec[:], pv[:, 64:65])
                qcg = 8 * qb + qc
                nc.any.tensor_scalar(
                    af_sb[:, pair, qcg, 64 * sub:64 * (sub + 1)], pv[:, 0:64],
                    rec[:], None, mybir.AluOpType.mult)
                if sub == 1:
                    nc.sync.dma_start_transpose(
                        afT_sb[:, pair, 128 * qcg:128 * (qcg + 1)],
                        af_sb[:, pair, qcg, :])

        def emit_tail(qb):
            for zc in range(8):
                ob = wrk.tile([128, 2, 512], BF, name="ob", tag="ob", bufs=6)
                for qf in range(2):
                    qsl = slice(1024 * qb + 512 * qf, 1024 * qb + 512 * (qf + 1))
                    if qb == 1 and (zc + qf) % 2 == 1:
                        fp = ps_big()[:, 0:512]
                    else:
                        fp = ps_small()
                    for c in range(2):
                        nc.tensor.matmul(fp[:], wfc_sb[:, c, zc, :],
                                         afT_sb[:, c, qsl],
                                         start=(c == 0), stop=(c == 1))
                    nc.any.tensor_copy(ob[:, qf, :], fp[:])
                nc.sync.dma_start(
                    outT[128 * zc:128 * (zc + 1), 1024 * qb:1024 * (qb + 1)],
                    ob[:])

        nc.sync.dma_start(ct_sb[:], ct2[:])
        nc.sync.dma_start(st_sb[:], st2[:])
        nc.sync.dma_start(wkv_sb[:], wkv.rearrange("(zc p) m -> p zc m", p=128))
        nc.sync.dma_start(wkr8_sb[:], wkr8.rearrange("(zp two p) m -> p zp two m", p=128, two=2))
        for _w in range(12):
            wps = ps_small()
            nc.tensor.matmul(wps[:], ct_sb[:, 0:128], ct_sb[:, 0:512],
                             start=True, stop=True)
        emit_kside(0)
        nc.sync.dma_start(wq8_sb[:], wq8.rearrange("(zp two p) m -> p zp two m", p=128, two=2))
        emit_qside(0)
        emit_qside(1)
        nc.sync.dma_start(wfc_sb[:], wfc.rearrange("(c p) (zc m) -> p c zc m", p=128, m=128))
        SCH0 = (1, 5, 9, 13)
        SCH1 = (2, 5, 7, 10, 13, 15)
        emit_scores(0, 0, range(4), SCH0)
        emit_scores(1, 0, range(4), SCH0)
        emit_kside(1)
        emit_scores(0, 0, range(4, 8), SCH0)
        emit_scores(1, 0, range(4, 8), SCH0)
        emit_kside(2)
        emit_kside(3)
        emit_scores(0, 0, range(8, 16), SCH0)
        emit_scores(1, 0, range(8, 16), SCH0)
        emit_qside(2)
        emit_qside(3)
        emit_pv(0, 0)
        emit_scores(2, 0, range(16), SCH1)
        emit_pv(1, 0)
        emit_scores(3, 0, range(16), SCH1)
        emit_pv(2, 0)
        emit_pv(3, 0)
        emit_tail(0)
        emit_scores(0, 1, range(16), SCH1)
        emit_pv(0, 1)
        emit_scores(1, 1, range(16), SCH1)
        emit_scores(2, 1, range(16), SCH1)
        emit_pv(1, 1)
        emit_scores(3, 1, range(16), SCH1)
        emit_pv(2, 1)
        emit_pv(3, 1)
        emit_tail(1)

    nc.compile()
    return nc


def prep_fast2(inputs):
    f32 = np.float32
    q = np.asarray(inputs["query"], f32)
    k = np.asarray(inputs["key"], f32)
    w_dq = np.asarray(inputs["w_dq"], f32)
    w_dkv = np.asarray(inputs["w_dkv"], f32)
    w_uq = np.asarray(inputs["w_uq"], f32)
    w_uk = np.asarray(inputs["w_uk"], f32)
    w_uv = np.asarray(inputs["w_uv"], f32)
    w_qr = np.asarray(inputs["w_qr"], f32)
    w_kr = np.asarray(inputs["w_kr"], f32)
    w_fc = np.asarray(inputs["w_fc"], f32)

    CT, ST = _rope_tables_f()
    ct2 = np.concatenate([CT, CT], axis=0).astype(bf16)
    st2 = np.concatenate([ST, ST], axis=0).astype(bf16)
    w_kr_p = _partner_cols(w_kr)
    wkr8 = np.concatenate([w_kr, w_kr, w_kr_p, w_kr_p], axis=1).astype(e4m3)

    WUK = w_dkv @ w_uk
    WUV = w_dkv @ w_uv
    WUQ = w_dq @ w_uq
    WQR = w_dq @ w_qr
    WQRP = _partner_cols(WQR)

    q8b = [q[b_].T.astype(e4m3) for b_ in range(B)]
    kTb = [k[b_].T.astype(bf16) for b_ in range(B)]
    k8b = [k[b_].T.astype(e4m3) for b_ in range(B)]

    in_maps = []
    for core in range(NCORES):
        b_idx, grp = core // HPC, core % HPC
        h0 = HPC * grp
        WKV = np.zeros((Z, 512), f32)
        WQ = np.zeros((Z, 768), f32)
        for p in range(2):
            ha, hb = h0 + 2 * p, h0 + 2 * p + 1
            for i, hh in enumerate((ha, hb)):
                csl = slice(64 * hh, 64 * hh + 64)
                WKV[:, 128 * p + 64 * i:128 * p + 64 * i + 64] = WUK[:, csl]
                WQ[:, 384 * p + 64 * i:384 * p + 64 * i + 64] = WUQ[:, csl]
                WQ[:, 384 * p + 128 + 64 * i:384 * p + 128 + 64 * i + 64] = WQR[:, csl]
                WQ[:, 384 * p + 256 + 64 * i:384 * p + 256 + 64 * i + 64] = WQRP[:, csl]
        hsl = slice(64 * h0, 64 * h0 + 256)
        WKV[:, 256:512] = WUV[:, hsl]
        in_maps.append({
            "q8": q8b[b_idx], "kT": kTb[b_idx], "k8": k8b[b_idx],
            "wkv": WKV.astype(bf16), "wq8": WQ.astype(e4m3),
            "wkr8": wkr8,
            "ct2": ct2, "st2": st2,
            "wfc": w_fc[hsl, :].astype(bf16),
        })
    return in_maps


def _partner_cols_f(w):
    return _partner_cols(w)


def _rope_tables_f():
    theta = 1.0 / (10000.0 ** (np.arange(0, RHD, 2, dtype=np.float32) / RHD))
    pe = np.arange(S, dtype=np.float32)[:, None] * theta[None, :]
    ct = np.repeat(np.sin(pe), 2, axis=-1).T.astype(np.float32)
    st = np.repeat(np.cos(pe), 2, axis=-1).T.astype(np.float32)
    return ct, st


def prep_fast(inputs):
    f32 = np.float32
    q = np.asarray(inputs["query"], f32)
    k = np.asarray(inputs["key"], f32)
    w_dq = np.asarray(inputs["w_dq"], f32)
    w_dkv = np.asarray(inputs["w_dkv"], f32)
    w_uq = np.asarray(inputs["w_uq"], f32)
    w_uk = np.asarray(inputs["w_uk"], f32)
    w_uv = np.asarray(inputs["w_uv"], f32)
    w_qr = np.asarray(inputs["w_qr"], f32)
    w_kr = np.asarray(inputs["w_kr"], f32)
    w_fc = np.asarray(inputs["w_fc"], f32)

    CT, ST = _rope_tables_f()
    ct2 = np.concatenate([CT, CT], axis=0).astype(bf16)
    st2 = np.concatenate([ST, ST], axis=0).astype(bf16)
    wkr48 = np.concatenate([_partner_cols_f(w_kr), w_kr, w_kr,
                            _partner_cols_f(w_kr)], axis=1).astype(e4m3)
    w_qr_p = _partner_cols_f(w_qr)

    q8b = [q[b_].T.astype(e4m3) for b_ in range(B)]
    kTb = [k[b_].T.astype(bf16) for b_ in range(B)]
    k8b = [k[b_].T.astype(e4m3) for b_ in range(B)]

    in_maps = []
    for core in range(NCORES):
        b_idx, grp = core // HPC, core % HPC
        h0 = HPC * grp
        WQ = np.zeros((DOWN, 768), f32)
        WK = np.zeros((DOWN, 256), f32)
        for p in range(2):
            ha, hb = h0 + 2 * p, h0 + 2 * p + 1
            for i, hh in enumerate((ha, hb)):
                csl = slice(64 * hh, 64 * hh + 64)
                WQ[:, 384 * p + 64 * i:384 * p + 64 * i + 64] = w_uq[:, csl]
                WQ[:, 384 * p + 128 + 64 * i:384 * p + 128 + 64 * i + 64] = w_qr[:, csl]
                WQ[:, 384 * p + 256 + 64 * i:384 * p + 256 + 64 * i + 64] = w_qr_p[:, csl]
                WK[:, 128 * p + 64 * i:128 * p + 64 * i + 64] = w_uk[:, csl]
        hsl = slice(64 * h0, 64 * h0 + 256)
        in_maps.append({
            "q8": q8b[b_idx], "kT": kTb[b_idx], "k8": k8b[b_idx],
            "wdq8": w_dq.astype(e4m3), "wdkv": w_dkv.astype(bf16),
            "wkr48": wkr48,
            "wq8": WQ.astype(e4m3), "wk": WK.astype(bf16),
            "wv": w_uv[:, hsl].astype(bf16),
            "ct2": ct2, "st2": st2,
            "wfc": w_fc[hsl, :].astype(bf16),
        })
    return in_maps


def post_fast(inputs, results):
    f32 = np.float32
    out = np.zeros((B, S, Z), f32)
    for core in range(NCORES):
        out[core // HPC] += np.asarray(results[core]["outT"], np.float32).T
    bias = (np.asarray(inputs["b_fc"], f32)
            + np.asarray(inputs["b_uv"], f32) @ np.asarray(inputs["w_fc"], f32))
    out += bias[None, None, :]
    return out.astype(f32)


def kernel(**inputs) -> np.ndarray:
    with_bias = any(np.any(np.asarray(inputs[n])) for n in
                    ("b_dq", "b_dkv", "b_uq", "b_uk", "b_qr", "b_kr"))
    if not with_bias:
        if "fast2" not in _cache:
            _cache["fast2"] = build_fast2()
        nc = _cache["fast2"]
        in_maps = prep_fast2(inputs)
        res = run_bass_kernel_spmd(nc, in_maps, core_ids=list(range(NCORES)))
        return post_fast(inputs, res.results)

    in_maps, _ = _prep_in_maps(inputs)
    key = ("nc", True)
    if key not in _cache:
        _cache[key] = build_nc(True)
    nc = _cache[key]
    res = run_bass_kernel_spmd(nc, in_maps, core_ids=list(range(NCORES)))
    f32 = np.float32
    out = np.zeros((B, S, Z), f32)
    for core in range(NCORES):
        out[core // HPC] += res.results[core]["outT"].T
    bias = (np.asarray(inputs["b_fc"], f32)
            + np.asarray(inputs["b_uv"], f32) @ np.asarray(inputs["w_fc"], f32))
    out += bias[None, None, :]
    return out.astype(np.float32)

